# revision 1
# baseline (speedup 1.0000x reference)
"""Trainium2 Bass kernel for nn_GroupPointEncoder.

Reference computation (G=4, B=8, N=2048, F=128):
  std = 2 or 4 per point by label class
  coords = [point_coord, (point_coord + noise*std)[1:]]           # [G,B,N,3]
  normed = (coords - low) / (high - low)
  pe     = interleaved sin/cos embedding, (y,x,z) order            # [G,B,N,384]
  h      = relu(pe @ W1.T + b1)                                    # [G,B,N,512]
  pos    = h @ W2.T + b2                                           # [G,B,N,256]
  query  = label_weight[labels] + pos
  out    = concat([query_pos, query], -1).reshape(G*B, N, 512)

Sharding: data-parallel over the G*B=32 (g,b) pairs, 4 per core, 8 cores.
Each core computes its 4*2048=8192 points' `query` half on device; the
query_pos half is a passthrough assembled on the host.

Device layout (feature-major): per 512-point tile
  arg[128,3,512](PSUM)  = outer(s_k, prescaled_coords)   3 K=1 matmuls
  range-reduce arg to [-pi,pi] via int32 round-trip      DVE/GpSimd
  pe[128,3,512]         = Sin(arg + bias_vec)            1 ACT op (bias 0 / pi/2)
  h[128,4,512]          = relu(W1p @ pe + b1)            12 f32r matmuls + DVE
  q[128,2,512]          = W2 @ h + onehot.T@(lab_w+b2)   10 f32r matmuls accum
"""
import sys
import math

sys.path.insert(0, "/opt/trn_rl_repo")

import numpy as np
from contextlib import ExitStack

import concourse.bass as bass
import concourse.tile as tile
from concourse import bacc, library_config, mybir
from concourse.bass_utils import run_bass_kernel_spmd

# problem constants (hardcoded per contract)
G, B, N, F = 4, 8, 2048, 128
NCORES = 8
BPC = B * G // NCORES          # 4 (g,b) pairs per core
NPTS = BPC * N                 # 8192 points per core
T = 512                        # points per tile
NT = NPTS // T                 # 16 tiles
TWO_PI = 2.0 * math.pi
INV_TWO_PI = 1.0 / TWO_PI
F32 = mybir.dt.float32
F32R = mybir.dt.float32r
I32 = mybir.dt.int32

_CACHE = {}


def _build_program():
    nc = bacc.Bacc("TRN2", target_bir_lowering=False, debug=False, num_devices=NCORES)

    pc_d = nc.dram_tensor("pc", [NT, 1, 3, T], F32, kind="ExternalInput").ap()
    oh_d = nc.dram_tensor("oh", [NT, 10, T], F32R, kind="ExternalInput").ap()
    w1t_d = nc.dram_tensor("w1t", [3, 128, 512], F32R, kind="ExternalInput").ap()
    w2t_d = nc.dram_tensor("w2t", [4, 128, 256], F32R, kind="ExternalInput").ap()
    lwb_d = nc.dram_tensor("lwb", [10, 256], F32R, kind="ExternalInput").ap()
    svec_d = nc.dram_tensor("svec", [128, 1], F32, kind="ExternalInput").ap()
    sdiv_d = nc.dram_tensor("sdiv", [128, 1], F32, kind="ExternalInput").ap()
    invs2_d = nc.dram_tensor("invs2", [128, 1], F32, kind="ExternalInput").ap()
    bvec_d = nc.dram_tensor("bvec", [128, 1], F32, kind="ExternalInput").ap()
    b1c_d = nc.dram_tensor("b1c", [128, 4], F32, kind="ExternalInput").ap()
    q_d = nc.dram_tensor("q", [256, NPTS], F32, kind="ExternalOutput").ap()

    with tile.TileContext(nc) as tc, ExitStack() as ctx:
        cpool = ctx.enter_context(tc.tile_pool(name="consts", bufs=1))
        wpool = ctx.enter_context(tc.tile_pool(name="weights", bufs=1))
        io = ctx.enter_context(tc.tile_pool(name="io", bufs=3))
        work = ctx.enter_context(tc.tile_pool(name="work", bufs=2))
        psum_h = ctx.enter_context(tc.tile_pool(name="ph", bufs=1, space="PSUM"))
        psum_q = ctx.enter_context(tc.tile_pool(name="pq", bufs=2, space="PSUM"))

        nc.gpsimd.load_library(library_config.proxy)
        svec = cpool.tile([128, 1], F32)
        nc.sync.dma_start(svec[:], svec_d[:])
        sdiv = cpool.tile([128, 1], F32)
        nc.sync.dma_start(sdiv[:], sdiv_d[:])
        invs2 = cpool.tile([128, 1], F32)
        nc.sync.dma_start(invs2[:], invs2_d[:])
        bvec = cpool.tile([128, 1], F32)
        nc.sync.dma_start(bvec[:], bvec_d[:])
        b1c = cpool.tile([128, 4], F32)
        nc.sync.dma_start(b1c[:], b1c_d[:])
        lwb = cpool.tile([10, 256], F32R)
        nc.sync.dma_start(lwb[:], lwb_d[:])

        w1t = []
        for k in range(3):
            w = wpool.tile([128, 512], F32R, name=f"w1t{k}", tag=f"w1t{k}")
            nc.sync.dma_start(w[:], w1t_d[k])
            w1t.append(w)
        w2t = []
        for k in range(4):
            w = wpool.tile([128, 256], F32R, name=f"w2t{k}", tag=f"w2t{k}")
            nc.sync.dma_start(w[:], w2t_d[k])
            w2t.append(w)

        for t in range(NT):
            pc_t = io.tile([1, 3, T], F32, tag="pc_t")
            nc.sync.dma_start(pc_t[:], pc_d[t])
            oh_t = io.tile([10, T], F32R, tag="oh_t")
            nc.sync.dma_start(oh_t[:], oh_d[t])

            # ---- stage 1: broadcast prescaled coords across partitions (exact f32)
            bc = work.tile([128, 3, T], F32, tag="bc")
            for c in range(3):
                nc.gpsimd.partition_broadcast(bc[:, c, :], pc_t[:, c, :])

            # ---- stage 2: phase reduction in coordinate space:
            #   ki = round(bc * s/2pi);  bc2 = bc - ki * 2pi/s
            # then arg = s*bc2 = s*bc - 2pi*ki  lands in [-pi, pi]
            ki = work.tile([128, 3, T], I32, tag="ki")
            nc.vector.tensor_scalar(ki[:], bc[:], sdiv[:], None, op0=mybir.AluOpType.mult)
            kf = work.tile([128, 3, T], F32, tag="kf")
            nc.vector.tensor_scalar(kf[:], ki[:], invs2[:], None, op0=mybir.AluOpType.mult)
            bc2 = work.tile([128, 3, T], F32, tag="bc2")
            nc.gpsimd.tensor_sub(bc2[:], bc[:], kf[:])

            # ---- stage 3: pe = sin(s*bc2 + bias)  (rows 0:64 sin, 64:128 cos)
            pe = work.tile([128, 3, T], F32R, tag="pe")
            nc.scalar.activation(
                pe[:],
                bc2[:],
                mybir.ActivationFunctionType.Sin,
                bias=bvec[:],
                scale=svec[:],
            )

            # ---- stage 4: h = relu(W1p @ pe + b1), feature-major [4x128, T]
            hp = psum_h.tile([128, 4, T], F32, tag="hp")
            for m in range(4):
                for k in range(3):
                    nc.tensor.matmul(
                        hp[:, m, :],
                        w1t[k][:, m * 128 : (m + 1) * 128],
                        pe[:, k, :],
                        start=(k == 0),
                        stop=(k == 2),
                    )
            h = work.tile([128, 4, T], F32R, tag="h")
            for m in range(4):
                nc.scalar.activation(
                    h[:, m, :],
                    hp[:, m, :],
                    mybir.ActivationFunctionType.Relu,
                    bias=b1c[:, m : m + 1],
                )

            # ---- stage 5: q = W2 @ h + onehot^T-gather, feature-major [2x128, T]
            for mp in range(2):
                qp = psum_q.tile([128, T], F32, tag="qp")
                for k in range(4):
                    nc.tensor.matmul(
                        qp[:],
                        w2t[k][:, mp * 128 : (mp + 1) * 128],
                        h[:, k, :],
                        start=(k == 0),
                        stop=False,
                    )
                nc.tensor.matmul(
                    qp[:],
                    lwb[:, mp * 128 : (mp + 1) * 128],
                    oh_t[:],
                    start=False,
                    stop=True,
                )
                qs = work.tile([128, T], F32, tag="qs")
                nc.vector.tensor_copy(qs[:], qp[:])
                nc.sync.dma_start(q_d[mp * 128 : (mp + 1) * 128, t * T : (t + 1) * T], qs[:])

    nc.compile()
    return nc


def _host_prep(point_coord, labels, pc_range, noise, label_weight, W1, b1, W2, b2):
    """Build the per-core input maps (host-side sharding + weight prep)."""
    pc32 = np.asarray(point_coord, np.float32)
    lab = np.asarray(labels)
    noi = np.asarray(noise, np.float32)
    rng = np.asarray(pc_range, np.float32)

    small = (lab == 0) | (lab >= 6)
    std = np.where(small, 2.0, 4.0).astype(np.float32)            # [B,N]
    coords = pc32[None] + noi * std[None, :, :, None]             # [G,B,N,3]
    coords[0] = pc32                                              # group 0 originals
    low, high = rng[:3], rng[3:]
    pcs = (coords - low) / (high - low) * np.float32(TWO_PI)      # [G,B,N,3]
    pcs = pcs[..., [1, 0, 2]]   # reference concatenates pe in (y,x,z) order
    onehot = np.eye(10, dtype=np.float32)[np.asarray(lab, np.int64)]  # [B,N,10]

    # feature permutation: kernel row c*128+k -> ref feature c*128+2k (sin),
    # row c*128+64+k -> c*128+2k+1 (cos)
    perm = np.empty(3 * F, np.int64)
    for c in range(3):
        for k in range(64):
            perm[c * 128 + k] = c * 128 + 2 * k
            perm[c * 128 + 64 + k] = c * 128 + 2 * k + 1
    w1p = np.ascontiguousarray(np.asarray(W1, np.float32)[:, perm].T)  # [384,512]
    w2t = np.ascontiguousarray(np.asarray(W2, np.float32).T)           # [512,256]
    lwb = np.asarray(label_weight, np.float32) + np.asarray(b2, np.float32)[None]
    b1c = np.ascontiguousarray(np.asarray(b1, np.float32).reshape(4, 128).T)

    k64 = np.arange(64, dtype=np.float64)
    s64 = 10000.0 ** (-k64 / 64.0)
    s128 = np.concatenate([s64, s64])
    svec = s128.astype(np.float32).reshape(128, 1)
    sdiv = (s128 / (2 * np.pi)).astype(np.float32).reshape(128, 1)
    invs2 = (2 * np.pi / s128).astype(np.float32).reshape(128, 1)
    bvec = np.concatenate(
        [np.zeros(64, np.float32), np.full(64, np.pi / 2, np.float32)]
    ).reshape(128, 1)

    shared = {
        "w1t": w1p.reshape(3, 128, 512),
        "w2t": w2t.reshape(4, 128, 256),
        "lwb": np.ascontiguousarray(lwb),
        "svec": np.ascontiguousarray(svec),
        "sdiv": np.ascontiguousarray(sdiv),
        "invs2": np.ascontiguousarray(invs2),
        "bvec": np.ascontiguousarray(bvec),
        "b1c": b1c,
    }

    in_maps = []
    for core in range(NCORES):
        g = core // 2
        b0 = 4 * (core % 2)
        # [4b, N, 3] -> [3, NPTS] -> [3, NT, T] -> [NT, 3, T]
        pcc = pcs[g, b0 : b0 + 4].reshape(NPTS, 3).T
        pcc = np.ascontiguousarray(pcc.reshape(3, NT, T).transpose(1, 0, 2)).reshape(
            NT, 1, 3, T
        )
        ohc = onehot[b0 : b0 + 4].reshape(NPTS, 10).T
        ohc = np.ascontiguousarray(ohc.reshape(10, NT, T).transpose(1, 0, 2))
        in_maps.append({"pc": pcc, "oh": ohc, **shared})
    return in_maps


def _get_nc():
    if "nc" not in _CACHE:
        _CACHE["nc"] = _build_program()
    return _CACHE["nc"]


def _run_device(in_maps, trace=False, **kw):
    nc = _get_nc()
    return run_bass_kernel_spmd(nc, in_maps, list(range(NCORES)), trace=trace, **kw)


def kernel(point_coord, labels, pc_range, noise, query_pos, label_weight, W1, b1, W2, b2):
    in_maps = _host_prep(
        point_coord, labels, pc_range, noise, label_weight, W1, b1, W2, b2
    )
    res = _run_device(in_maps)

    qp = np.asarray(query_pos, np.float32)
    out = np.empty((G * B, N, 4 * F), np.float32)
    out[:, :, : 2 * F] = qp.reshape(G * B, N, 2 * F)
    for core in range(NCORES):
        q = res.results[core]["q"]                       # [256, NPTS]
        q = q.reshape(2 * F, BPC, N).transpose(1, 2, 0)  # [4, N, 256]
        out[4 * core : 4 * core + 4, :, 2 * F :] = q
    return out



# revision 3
# speedup vs baseline: 2.4082x; 2.4082x over previous
"""Trainium2 Bass kernel for nn_GroupPointEncoder.

Reference computation (G=4, B=8, N=2048, F=128):
  std = 2 or 4 per point by label class
  coords = [point_coord, (point_coord + noise*std)[1:]]           # [G,B,N,3]
  normed = (coords - low) / (high - low)
  pe     = interleaved sin/cos embedding, (y,x,z) order            # [G,B,N,384]
  h      = relu(pe @ W1.T + b1)                                    # [G,B,N,512]
  pos    = h @ W2.T + b2                                           # [G,B,N,256]
  query  = label_weight[labels] + pos
  out    = concat([query_pos, query], -1).reshape(G*B, N, 512)

Sharding: data-parallel over the G*B=32 (g,b) pairs, 4 per core, 8 cores.
Each core computes its 4*2048=8192 points' `query` half on device; the
query_pos half is a passthrough assembled on the host.

Device layout (feature-major, fp16 matmul path; all matmuls 1 cycle/row):
  partition rows per coord chunk: p<48 sin(s_{16+p} x), 48<=p<96
  cos(s_{p-32} x), 96<=p<128 "hot" rows (freqs 0..15) whose args can
  leave the Sin table domain [-pi,pi]; those sin/cos values are computed
  exactly on the host and DMA'd straight into the pe tile.  Cold rows:
  one ACT Sin op per tile, pe[0:96] = sin(svec*bc + bvec), where bc is
  the host-replicated coordinate tile (no on-device broadcast, no range
  reduction -> GpSimd idle, DVE only does relu).
  h[128,4,T](PSUM halves) = relu(W1p @ pe + b1)   12 fp16 matmuls + DVE
  q[128,2,T]              = W2 @ h + onehot.T@(lab_w+b2)  10 fp16 matmuls
  PE queue is software-pipelined: s4(t), s5(t-1), s4(t+1), ... so the
  tensor engine never waits on relu; ACT drains PSUM two tiles behind.
"""
import sys
import math
from collections import deque

sys.path.insert(0, "/opt/trn_rl_repo")

import numpy as np
from contextlib import ExitStack

import concourse.bass as bass
import concourse.tile as tile
from concourse import bacc, mybir
from concourse.bass_utils import run_bass_kernel_spmd

# problem constants (hardcoded per contract)
G, B, N, F = 4, 8, 2048, 128
NCORES = 8
BPC = B * G // NCORES          # 4 (g,b) pairs per core
NPTS = BPC * N                 # 8192 points per core
T = 512                        # points per tile
NT = NPTS // T                 # 16 tiles
HOT = 28                       # freqs 0..27 need exact (host) reduction
COLD = 128 - 2 * HOT           # 72 cold rows (sin 36 + cos 36, freqs 28..63)
TWO_PI = 2.0 * math.pi
F32 = mybir.dt.float32
F16 = mybir.dt.float16

_CACHE = {}


def _build_program():
    nc = bacc.Bacc("TRN2", target_bir_lowering=False, debug=False, num_devices=NCORES)

    bc_d = nc.dram_tensor("bc", [NT, COLD, 3, T], F16, kind="ExternalInput").ap()
    peh_d = nc.dram_tensor("peh", [NT, 2 * HOT, 3, T], F16, kind="ExternalInput").ap()
    oh_d = nc.dram_tensor("oh", [NT, 10, T], F16, kind="ExternalInput").ap()
    w1t_d = nc.dram_tensor("w1t", [3, 128, 512], F16, kind="ExternalInput").ap()
    w2t_d = nc.dram_tensor("w2t", [4, 128, 256], F16, kind="ExternalInput").ap()
    lwb_d = nc.dram_tensor("lwb", [10, 256], F16, kind="ExternalInput").ap()
    svec_d = nc.dram_tensor("svec", [128, 1], F32, kind="ExternalInput").ap()
    bvec_d = nc.dram_tensor("bvec", [128, 1], F32, kind="ExternalInput").ap()
    b1c_d = nc.dram_tensor("b1c", [128, 4], F32, kind="ExternalInput").ap()
    q_d = nc.dram_tensor("q", [NT, 256, T], F16, kind="ExternalOutput").ap()

    with tile.TileContext(nc) as tc, ExitStack() as ctx:
        cpool = ctx.enter_context(tc.tile_pool(name="consts", bufs=1))
        wpool = ctx.enter_context(tc.tile_pool(name="weights", bufs=1))
        io = ctx.enter_context(tc.tile_pool(name="io", bufs=4))
        pepool = ctx.enter_context(tc.tile_pool(name="pe", bufs=3))
        hpool = ctx.enter_context(tc.tile_pool(name="h", bufs=2))
        qpool = ctx.enter_context(tc.tile_pool(name="qs", bufs=4))
        psum_h = ctx.enter_context(tc.tile_pool(name="ph", bufs=2, space="PSUM"))
        psum_q = ctx.enter_context(tc.tile_pool(name="pq", bufs=4, space="PSUM"))

        svec = cpool.tile([128, 1], F32)
        nc.sync.dma_start(svec[:], svec_d[:])
        bvec = cpool.tile([128, 1], F32)
        nc.sync.dma_start(bvec[:], bvec_d[:])
        b1c = cpool.tile([128, 4], F32)
        nc.sync.dma_start(b1c[:], b1c_d[:])
        lwb = cpool.tile([10, 256], F16)
        nc.sync.dma_start(lwb[:], lwb_d[:])

        w1t = []
        for k in range(3):
            w = wpool.tile([128, 512], F16, name=f"w1t{k}", tag=f"w1t{k}")
            nc.sync.dma_start(w[:], w1t_d[k])
            w1t.append(w)
        w2t = []
        for k in range(4):
            w = wpool.tile([128, 256], F16, name=f"w2t{k}", tag=f"w2t{k}")
            nc.sync.dma_start(w[:], w2t_d[k])
            w2t.append(w)

        pend5 = deque()   # (pe consumed; h_t, oh_t, t) awaiting stage-5
        pendc = deque()   # (qp0, qp1, t) awaiting PSUM drain + DMA out

        def emit_stage5(h_t, oh_t, t):
            qps = []
            for mp in range(2):
                qp = psum_q.tile([128, T], F32, tag="qp")
                for k in range(4):
                    nc.tensor.matmul(
                        qp[:],
                        w2t[k][:, mp * 128 : (mp + 1) * 128],
                        h_t[:, k, :],
                        start=(k == 0),
                        stop=False,
                    )
                nc.tensor.matmul(
                    qp[:],
                    lwb[:, mp * 128 : (mp + 1) * 128],
                    oh_t[:],
                    start=False,
                    stop=True,
                )
                qps.append(qp)
            pendc.append((qps[0], qps[1], t))

        def emit_drain():
            qp0, qp1, t = pendc.popleft()
            for mp, qp in ((0, qp0), (1, qp1)):
                qs = qpool.tile([128, T], F16, tag="qs")
                nc.scalar.copy(qs[:], qp[:])
                nc.sync.dma_start(q_d[t, mp * 128 : (mp + 1) * 128, :], qs[:])

        for t in range(NT):
            # ---- input DMAs (prefetched via pool bufs)
            bc_t = io.tile([COLD, 3, T], F16, tag="bc_t")
            nc.sync.dma_start(bc_t[:], bc_d[t])
            oh_t = io.tile([10, T], F16, tag="oh_t")
            nc.sync.dma_start(oh_t[:], oh_d[t])
            pe_t = pepool.tile([128, 3, T], F16, tag="pe_t")
            nc.sync.dma_start(pe_t[COLD:128, :, :], peh_d[t])

            # ---- cold rows: pe[0:96] = sin(svec*bc + bvec)  (one ACT op)
            nc.scalar.activation(
                pe_t[0:COLD, :, :],
                bc_t[:],
                mybir.ActivationFunctionType.Sin,
                bias=bvec[0:COLD, :],
                scale=svec[0:COLD, :],
            )
            # ---- drain qp of tile t-2 (ACT, after sin so it never blocks it)
            if len(pendc) >= 2:
                emit_drain()

            # ---- stage 4: h = relu(W1p @ pe + b1), two PSUM halves
            hps = []
            for half in range(2):
                hp = psum_h.tile([128, 2, T], F32, tag="hp")
                for mh in range(2):
                    m = half * 2 + mh
                    for k in range(3):
                        nc.tensor.matmul(
                            hp[:, mh, :],
                            w1t[k][:, m * 128 : (m + 1) * 128],
                            pe_t[:, k, :],
                            start=(k == 0),
                            stop=(k == 2),
                        )
                hps.append(hp)
            h_t = hpool.tile([128, 4, T], F16, tag="h_t")
            for half in range(2):
                for mh in range(2):
                    m = half * 2 + mh
                    nc.vector.tensor_scalar(
                        h_t[:, m, :],
                        hps[half][:, mh, :],
                        b1c[:, m : m + 1],
                        0.0,
                        op0=mybir.AluOpType.add,
                        op1=mybir.AluOpType.max,
                    )

            # ---- stage 5 of the previous tile (keeps PE stream gapless)
            if pend5:
                emit_stage5(*pend5.popleft())
            pend5.append((h_t, oh_t, t))

        while pend5:
            emit_stage5(*pend5.popleft())
        while pendc:
            emit_drain()

    nc.compile()
    return nc


def _host_prep(point_coord, labels, pc_range, noise, label_weight, W1, b1, W2, b2):
    """Build the per-core input maps (host-side sharding + weight prep)."""
    pc32 = np.asarray(point_coord, np.float32)
    lab = np.asarray(labels, np.int64)
    noi = np.asarray(noise, np.float32)
    rng = np.asarray(pc_range, np.float32)

    small = (lab == 0) | (lab >= 6)
    std = np.where(small, 2.0, 4.0).astype(np.float32)            # [B,N]
    coords = pc32[None] + noi * std[None, :, :, None]             # [G,B,N,3]
    coords[0] = pc32                                              # group 0 originals
    low, high = rng[:3], rng[3:]
    pcs = (coords - low) / (high - low) * np.float32(TWO_PI)      # [G,B,N,3]
    pcs = pcs[..., [1, 0, 2]]   # reference concatenates pe in (y,x,z) order
    onehot = np.eye(10, dtype=np.float16)[lab]                    # [B,N,10]

    # partition layout (C=COLD//2=36, H=HOT=28):
    #   p<C sin k=HOT+p, C<=p<2C cos k=HOT+(p-C)  (cold, on-device Sin)
    #   2C<=p<2C+H sin k=p-2C, then cos k=p-2C-H  (hot, host-computed)
    k64 = np.arange(64, dtype=np.float64)
    s64 = 10000.0 ** (-k64 / 64.0)
    C = COLD // 2
    fmap = np.empty(128, np.int64)
    svec = np.zeros(128, np.float64)
    bvec = np.zeros(128, np.float64)
    fmap[0:C] = 2 * (HOT + np.arange(C))
    fmap[C : 2 * C] = 2 * (HOT + np.arange(C)) + 1
    fmap[2 * C : 2 * C + HOT] = 2 * np.arange(HOT)
    fmap[2 * C + HOT : 128] = 2 * np.arange(HOT) + 1
    svec[0:C] = s64[HOT:]
    svec[C : 2 * C] = s64[HOT:]
    bvec[C : 2 * C] = math.pi / 2
    perm = (np.arange(3)[:, None] * 128 + fmap[None, :]).reshape(-1)

    w1p = np.ascontiguousarray(np.asarray(W1, np.float32)[:, perm].T)  # [384,512]
    w2t = np.ascontiguousarray(np.asarray(W2, np.float32).T)           # [512,256]
    lwb = np.asarray(label_weight, np.float32) + np.asarray(b2, np.float32)[None]
    b1c = np.ascontiguousarray(np.asarray(b1, np.float32).reshape(4, 128).T)

    shared = {
        "w1t": w1p.reshape(3, 128, 512).astype(np.float16),
        "w2t": w2t.reshape(4, 128, 256).astype(np.float16),
        "lwb": np.ascontiguousarray(lwb).astype(np.float16),
        "svec": np.ascontiguousarray(svec.reshape(128, 1)).astype(np.float32),
        "bvec": np.ascontiguousarray(bvec.reshape(128, 1)).astype(np.float32),
        "b1c": b1c,
    }
    sh = s64[:HOT].astype(np.float32)

    in_maps = []
    for core in range(NCORES):
        g = core // 2
        b0 = 4 * (core % 2)
        # [4b, N, 3] -> [3, NT, T]
        x3 = pcs[g, b0 : b0 + 4].reshape(NPTS, 3).T.reshape(3, NT, T)
        bcc = np.ascontiguousarray(
            np.broadcast_to(x3[None], (COLD, 3, NT, T)).transpose(2, 0, 1, 3)
        ).astype(np.float16)                                      # [NT, COLD, 3, T]
        ph = sh[:, None, None, None] * x3[None]                   # [HOT, 3, NT, T]
        peh = np.concatenate([np.sin(ph), np.cos(ph)], axis=0)    # [2*HOT, 3, NT, T]
        peh = np.ascontiguousarray(peh.transpose(2, 0, 1, 3)).astype(np.float16)
        ohc = onehot[b0 : b0 + 4].reshape(NPTS, 10).T
        ohc = np.ascontiguousarray(ohc.reshape(10, NT, T).transpose(1, 0, 2))
        in_maps.append({"bc": bcc, "peh": peh, "oh": ohc, **shared})
    return in_maps


def _get_nc():
    if "nc" not in _CACHE:
        _CACHE["nc"] = _build_program()
    return _CACHE["nc"]


def _run_device(in_maps, trace=False, **kw):
    nc = _get_nc()
    return run_bass_kernel_spmd(nc, in_maps, list(range(NCORES)), trace=trace, **kw)


def kernel(point_coord, labels, pc_range, noise, query_pos, label_weight, W1, b1, W2, b2):
    in_maps = _host_prep(
        point_coord, labels, pc_range, noise, label_weight, W1, b1, W2, b2
    )
    res = _run_device(in_maps)

    qp = np.asarray(query_pos, np.float32)
    out = np.empty((G * B, N, 4 * F), np.float32)
    out[:, :, : 2 * F] = qp.reshape(G * B, N, 2 * F)
    for core in range(NCORES):
        q = res.results[core]["q"]                       # [NT, 256, T] f16
        q = q.transpose(1, 0, 2).reshape(2 * F, BPC, N).transpose(1, 2, 0)
        out[4 * core : 4 * core + 4, :, 2 * F :] = q.astype(np.float32)
    return out
